# revision 48
# baseline (speedup 1.0000x reference)
"""Equivariant MLP (9 -> 49 -> 49 -> 9, tied weights) on 8 trn2 NeuronCores.

Data parallel over the batch (1048576 rows -> 131072/core).  Tied-weight
patterns are expanded to dense matrices on the host.  Samples are processed
in PAIRS: x^T arrives as [19, 65536] bf16 (rows 0-17 = two samples' features
stacked, row 18 = ones) and every layer's bias is folded into the matmul via
the ones row, which each weight matrix propagates (extra unit column) so no
engine ever adds a bias.

Per 1024-pair iteration:
  L1  PE   [19,99]w  x [19,1024]   -> psum1 [99,1024]   (bias via ones row)
  h1  ACT  relu(psum1) -> sbuf bf16 [99,1024]           (ones row survives)
  L2  PE   [99,99]w x h1           -> psum2 [99,1024]
  h2  DVE  max(psum2,0) -> sbuf bf16 [99,1024]
  L3  PE   FLIPPED: stationary = h2 128-col chunk, moving = [99,18] weights
           -> psum3 [128 pairs, 18] per chunk; 8 chunks = [128,144].
           Ldweights is free, so L3 costs 144 PE columns instead of 1024.
  y   ACT  copy psum3 -> sbuf f32, DMA out every 8 iterations.

Emission is software-pipelined (stages: L1 at i, h1/L2/h2 at i-1, L3 at
i-4, y at i-5) so no in-order engine queue waits behind a dependent op.
PSUM (8 banks): p1/p2 share a 3-slot rotation (6 banks); p3 has its own
2-slot tag (1 bank each) holding y for [3,3,2]-iteration groups so ACT
does only 3 y-copies per 8 iterations.  All x chunks are preloaded into
dedicated SBUF buffers; the first 512 x columns ride in the weight DMA so
the first matmul starts one DMA-latency after launch; junk matmuls during
that window pre-ramp the PE out of its low p-state.  Tail: the last h2
runs on the (by then idle, faster) ACT and the final y group flushes
per-iteration so the closing DMA is half-size.

TimelineSim: 92.2us/core (baseline 149.4us).  The steady state is bound
by ACT throughput: relu-evac 1038ns + ~57ns depth-0 SEQ dispatch + ~190ns
amortized y-copy per 1024-pair round; DVE runs 1192ns, PE ~930ns.
"""

import os
import sys

sys.path.insert(0, "/opt/trn_rl_repo")

import numpy as np
import ml_dtypes

import concourse.bass as bass
import concourse.mybir as mybir
import concourse.tile as tile
from concourse.bass_utils import run_bass_kernel_spmd

f32 = mybir.dt.float32
bf16 = mybir.dt.bfloat16

N_CORES = 8
BATCH = 1048576
BS = BATCH // N_CORES          # 131072 samples per core
NPAIR = BS // 2                # 65536 pair columns per core
C = 1024                       # pair columns per iteration
NITER = NPAIR // C             # 64
XCH = 8                        # iterations per x DMA chunk
YCH = 8                        # iterations per y DMA chunk
MM = 512                       # matmul moving width (one PSUM bank)

last_exec_ns = None


def _split_multi_waits(nc):
    """Walrus in this container rejects instructions carrying more than one
    sync wait ("Too many sync wait commands").  Re-park all but one wait of
    every instruction on same-engine NoOps inserted just before it."""
    n = 0
    for fn in nc.m.functions:
        for bb in fn.blocks:
            out = []
            for inst in bb.instructions:
                si = inst.sync_info
                waits = list(si.on_wait) if (si and si.on_wait) else []
                if len(waits) > 1:
                    si.on_wait = waits[-1:]
                    for w in waits[:-1]:
                        nop = mybir.InstNoOp(name=f"WSPLIT-{n}", ins=[], outs=[])
                        n += 1
                        nop.engine = inst.engine
                        nop.sync_info = mybir.SyncInfo(on_update=[], on_wait=[w])
                        out.append(nop)
                out.append(inst)
            bb.instructions = out
    return nc


def _build_nc():
    nc = bass.Bass()
    xt = nc.dram_tensor("xt", [19, NPAIR], bf16, kind="ExternalInput")
    # wp cols 0:99 = w2e, 99:117 = w3e, 117:216 = w1e (rows 0:19),
    # 216:216+MM = first 512 x columns (rows 0:19) — one startup DMA
    wp = nc.dram_tensor("wp", [99, 216 + MM], bf16, kind="ExternalInput")
    yt = nc.dram_tensor("yt", [NITER // YCH, 128, YCH * 144], f32,
                        kind="ExternalOutput")

    relu = mybir.ActivationFunctionType.Relu
    amax = mybir.AluOpType.max
    XW = XCH * C                   # pair columns per x chunk

    with tile.TileContext(nc) as tc:
        with (
            tc.tile_pool(name="consts", bufs=1) as cp,
            tc.tile_pool(name="xp", bufs=2) as xp,
            tc.tile_pool(name="hid", bufs=2) as hp,
            tc.tile_pool(name="yp", bufs=2) as yp,
            tc.tile_pool(name="ps", bufs=3, space=bass.MemorySpace.PSUM) as pp,
        ):
            wpt = cp.tile([99, 216 + MM], bf16)
            w2t = wpt[:, 0:99]
            w3t = wpt[:, 99:117]
            w1t = wpt[0:19, 117:216]

            xts = {}

            def xdma(g):
                xts[g] = xp.tile([19, XW], bf16, tag="xts",
                                 bufs=NITER // XCH, name=f"xts{g}")
                if g == 0:
                    # first 512 x cols ride in the wp DMA; a small DMA brings
                    # iteration 0's second half early, then the bulk
                    nc.sync.dma_start(xts[g][:, MM:C], xt[:, MM:C])
                    nc.sync.dma_start(xts[g][:, C:XW], xt[:, C:XW])
                else:
                    nc.sync.dma_start(xts[g][:], xt[:, g * XW:(g + 1) * XW])

            nc.sync.dma_start(wpt[:], wp[:])
            for g in range(NITER // XCH):
                xdma(g)

            # Warm the PE p-state during the initial DMA wait: junk matmuls
            # on a memset tile, output into the first p3 slot (reused by the
            # real L3 two slots later, safely behind these).
            junk = cp.tile([1, MM], bf16)
            nc.vector.memset(junk[:], 0.0)
            jp = pp.tile([128, 432], f32, tag="p3", bufs=2, name="jp")
            for _ in range(7):
                nc.tensor.matmul(jp[0:1, 0:192], junk[0:1, 0:1],
                                 junk[0:1, 0:192], start=True, stop=True)

            p1s, p2s, p3s, h1s, h2s = {}, {}, {}, {}, {}
            ycur = [None]
            ylast = [None]

            for i in range(NITER + 6):
                if i < NITER:                       # L1(i) on PE
                    p1 = pp.tile([99, C], f32, tag="ps", name=f"p1_{i}")
                    if i == 0:
                        nc.tensor.matmul(p1[:, 0:MM], w1t[:],
                                         wpt[0:19, 216:216 + MM],
                                         start=True, stop=True)
                        nc.tensor.matmul(p1[:, MM:C], w1t[:],
                                         xts[0][:, MM:C],
                                         start=True, stop=True)
                    else:
                        src = xts[i // XCH]
                        off = (i % XCH) * C
                        for m in range(0, C, MM):
                            nc.tensor.matmul(
                                p1[:, m:m + MM], w1t[:],
                                src[:, off + m:off + m + MM],
                                start=True, stop=True)
                    p1s[i] = p1

                if 1 <= i <= NITER:                 # h1(i-1) on ACT
                    k = i - 1
                    h1 = hp.tile([99, C], bf16, tag="h1", name=f"h1_{k}")
                    nc.scalar.activation(h1[:], p1s.pop(k)[:], relu)
                    h1s[k] = h1

                if 1 <= i <= NITER:                 # L2(i-1) on PE
                    k = i - 1
                    p2 = pp.tile([99, C], f32, tag="ps", name=f"p2_{k}")
                    h1 = h1s.pop(k)
                    for m in range(0, C, MM):
                        nc.tensor.matmul(
                            p2[:, m:m + MM], w2t[:], h1[:, m:m + MM],
                            start=True, stop=True)
                    p2s[k] = p2

                if 1 <= i <= NITER:                 # h2(i-1) on DVE, same round
                    k = i - 1
                    h2 = hp.tile([99, C], bf16, tag="h2", bufs=4,
                                 name=f"h2_{k}")
                    p2 = p2s.pop(k)
                    if k == NITER - 1:
                        # tail: ACT is idle after its last h1 and is faster
                        nc.scalar.activation(h2[:], p2[:], relu)
                    else:
                        nc.vector.tensor_scalar(h2[:], p2[:], 0.0, None,
                                                amax)
                    h2s[k] = h2

                # y iterations grouped [3,3,2] per 8-iter chunk: a PSUM bank
                # holds 3 iterations' worth ([128, 432]), so 3 copies/chunk.
                GSTART = {0: 0, 3: 1, 6: 2}          # k%8 -> group index
                GLAST = {2: 0, 5: 1, 7: 2}           # k%8 -> group index
                GW = (432, 432, 288)                 # group widths (cols)

                if 4 <= i <= NITER + 3:             # L3(i-4) on PE, flipped
                    k = i - 4
                    q = k % YCH
                    if q in GSTART:
                        gid = (k // YCH) * 3 + GSTART[q]
                        p3s[gid] = pp.tile([128, 432], f32, tag="p3",
                                           bufs=2, name=f"p3_{gid}")
                    gid = (k // YCH) * 3 + [g for r, g in GSTART.items()
                                            if r <= q][-1]
                    p3 = p3s[gid]
                    h2 = h2s.pop(k)
                    base = (q - [r for r in GSTART if r <= q][-1]) * 144
                    for c in range(8):
                        nc.tensor.matmul(
                            p3[:, base + c * 18:base + (c + 1) * 18],
                            h2[:, c * 128:(c + 1) * 128], w3t[:],
                            start=True, stop=True)
                    if k // YCH == NITER // YCH - 1 and q in (2, 5, 6, 7):
                        # tail: copy + DMA per completed piece; the final
                        # group flushes per-iteration so the last DMA is half
                        if ylast[0] is None:
                            ylast[0] = yp.tile([128, YCH * 144], f32,
                                               tag="yt", bufs=NITER // YCH,
                                               name="ylast")
                        if q in (2, 5):
                            go = GLAST[q]
                            c0, w = (0, 432, 864)[go], 432
                            src = p3s.pop(gid)[:, 0:w]
                        elif q == 6:
                            c0, w = 864, 144
                            src = p3s[gid][:, 0:144]
                        else:
                            c0, w = 1008, 144
                            src = p3s.pop(gid)[:, 144:288]
                        dst = ylast[0][:, c0:c0 + w]
                        nc.scalar.copy(dst, src)
                        nc.sync.dma_start(yt[k // YCH, :, c0:c0 + w], dst)

                if 5 <= i <= NITER + 4 and (i - 5) % YCH in GLAST:
                    k = i - 5                       # y copy for finished group
                    m = k // YCH
                    if m == NITER // YCH - 1:
                        pass                        # tail handled in L3 block
                    else:
                        go = GLAST[k % YCH]
                        gid = m * 3 + go
                        w = GW[go]
                        if go == 0:
                            ycur[0] = yp.tile([128, YCH * 144], f32,
                                              tag="yt", bufs=NITER // YCH,
                                              name=f"y_{m}")
                        c0 = (0, 432, 864)[go]
                        dst = ycur[0][:, c0:c0 + w]
                        nc.scalar.copy(dst, p3s.pop(gid)[:, 0:w])
                        if go == 2:
                            nc.sync.dma_start(yt[m], ycur[0][:])

    return _split_multi_waits(nc)


_nc_cache = {}


def _get_nc(*_ignored):
    if "nc" not in _nc_cache:
        _nc_cache["nc"] = _build_nc()
    return _nc_cache["nc"]


def _expand(pattern, params):
    pattern = np.asarray(pattern)
    params = np.asarray(params, np.float32)
    return np.where(pattern > 0, params[np.maximum(pattern - 1, 0)], 0.0).astype(
        np.float32
    )


def _blockdiag(a):
    o = np.zeros((2 * a.shape[0], 2 * a.shape[1]), np.float32)
    o[:a.shape[0], :a.shape[1]] = a
    o[a.shape[0]:, a.shape[1]:] = a
    return o


def kernel(**inputs):
    global last_exec_ns
    x = np.ascontiguousarray(np.asarray(inputs["x"], np.float32))
    W1 = _expand(inputs["wp1"], inputs["w1"])  # [9, 49]
    W2 = _expand(inputs["wp2"], inputs["w2"])  # [49, 49]
    W3 = _expand(inputs["wp3"], inputs["w3"])  # [49, 9]
    B1 = _expand(inputs["bp1"], inputs["b1"])  # [49]
    B2 = _expand(inputs["bp2"], inputs["b2"])  # [49]
    B3 = _expand(inputs["bp3"], inputs["b3"])  # [9]

    w1e = np.zeros((19, 99), np.float32)
    w1e[0:18, 0:98] = _blockdiag(W1)
    w1e[18, 0:98] = np.concatenate([B1, B1])
    w1e[18, 98] = 1.0
    w2e = np.zeros((99, 99), np.float32)
    w2e[0:98, 0:98] = _blockdiag(W2)
    w2e[98, 0:98] = np.concatenate([B2, B2])
    w2e[98, 98] = 1.0
    w3e = np.zeros((99, 18), np.float32)
    w3e[0:98, :] = _blockdiag(W3)
    w3e[98, :] = np.concatenate([B3, B3])

    wpk = np.zeros((99, 216 + MM), np.float32)
    wpk[:, 0:99] = w2e
    wpk[:, 99:117] = w3e
    wpk[0:19, 117:216] = w1e

    ones = np.ones((1, NPAIR), np.float32)
    in_maps = []
    for c in range(N_CORES):
        xc = x[c * BS:(c + 1) * BS]                       # [BS, 9]
        xpair = xc.reshape(NPAIR, 18).T                   # [18, NPAIR] view
        xfull = np.concatenate([xpair, ones], axis=0)     # [19, NPAIR]
        wpc = wpk.copy()
        wpc[0:19, 216:216 + MM] = xfull[:, 0:MM]
        in_maps.append({
            "xt": np.ascontiguousarray(xfull).astype(ml_dtypes.bfloat16),
            "wp": wpc.astype(ml_dtypes.bfloat16),
        })

    nc = _get_nc()
    trace = os.environ.get("KERNEL_TRACE", "0") == "1"
    # The axon-tunneled NRT intermittently fails with
    # NRT_EXEC_UNIT_UNRECOVERABLE; a plain retry recovers it.
    last_err = None
    for attempt in range(4):
        try:
            res = run_bass_kernel_spmd(
                nc, in_maps, core_ids=list(range(N_CORES)), trace=trace
            )
            break
        except Exception as e:  # noqa: BLE001
            last_err = e
            import time as _time

            _time.sleep(2.0 * (attempt + 1))
    else:
        raise last_err
    if trace:
        last_exec_ns = res.exec_time_ns

    y = np.empty((BATCH, 9), np.float32)
    for c in range(N_CORES):
        ytc = res.results[c]["yt"]  # [8, 128, 1152]
        # ytc[g, n, q*144 + cc*18 + h*9 + f] -> sample 2*(((g*8+q)*8+cc)*128+n)+h
        arr = ytc.reshape(8, 128, YCH, 8, 2, 9).transpose(0, 2, 3, 1, 4, 5)
        y[c * BS:(c + 1) * BS] = arr.reshape(BS, 9)
    return y
